# revision 1
# baseline (speedup 1.0000x reference)
"""ArcFace loss on 8 TRN2 NeuronCores — class-parallel (tensor-parallel classifier).

Full inputs in, full output out. Inside: shard the 100000-class weight matrix
across 8 cores (12500 rows each, zero-padded to 12544 = 98*128), replicate
features/labels, run one SPMD Bass kernel that computes a distributed
softmax-cross-entropy via a single 12KB AllReduce of per-row partial
sums, and return the scalar loss.

Key numeric facts exploited:
  - logits = 64*cos(theta) are bounded by 64 -> exp(logits) never overflows
    fp32, so no per-row max pass / max all-reduce is needed.
  - rsqrt is computed as exp(-0.5*ln(x)) so the whole kernel uses a single
    ACT table set (natural_log_exp_and_others); Sqrt would force ~2.7us
    table switches inside the loop.
  - margin correction: sumexp' = sumexp + exp(g-32) - exp(g) where
    g = 64*cos at the label column, computed via an indirect-DMA gather of
    the label rows; masked so only the owning core contributes.
"""

import numpy as np

import concourse.bass as bass
import concourse.mybir as mybir
import concourse.tile as tile
from concourse import bacc
from concourse.bass import ts
from concourse.masks import make_identity

F32 = mybir.dt.float32
BF16 = mybir.dt.bfloat16
FP8 = mybir.dt.float8e4
I32 = mybir.dt.int32
AF = mybir.ActivationFunctionType
ALU = mybir.AluOpType

P = 128
B = 1024          # batch
D = 512           # feature dim
C = 100000        # classes
NCORE = 8
CS = C // NCORE   # 12500 per-core classes
CS_PAD = 12544    # 98 * 128
NBT = B // P      # 8 b-tiles
NK = D // P       # 4 k-chunks
CHUNK = 512
NCHUNK = (CS_PAD + CHUNK - 1) // CHUNK  # 25 (24 full + 1 of 256)
SCALE = 64.0
SM = SCALE * 0.5  # scale*margin = 32


def main_chunk_loop(nc, tc, wd, wp, pst, psm, wsh, fT, ident, srows, n_chunks):
    for ci in range(n_chunks):
        c0 = ci * CHUNK
        csz = min(CHUNK, CS_PAD - c0)
        nsub = csz // P

        wnat = wd.tile([P, 4, D], F32, name="wnat", tag="wnat")
        nc.sync.dma_start(
            out=wnat[:, :nsub, :],
            in_=wsh[c0 : c0 + csz, :].rearrange("(s p) d -> p s d", p=P),
        )
        n2 = wp.tile([P, 4], F32, name="n2", tag="n2")
        for i in range(nsub):
            sq = wp.tile([P, D], F32, name="sq", tag="sqdump")
            nc.vector.scalar_tensor_tensor(
                out=sq[:],
                in0=wnat[:, i, :],
                scalar=1.0,
                in1=wnat[:, i, :],
                op0=ALU.mult,
                op1=ALU.mult,
                accum_out=n2[:, i : i + 1],
            )
        wrn = wp.tile([P, 4], F32, name="wrn", tag="wrn")
        nc.vector.tensor_scalar_add(wrn[:, :nsub], n2[:, :nsub], 1e-24)
        nc.scalar.activation(out=wrn[:, :nsub], in_=wrn[:, :nsub], func=AF.Ln)
        nc.scalar.activation(
            out=wrn[:, :nsub], in_=wrn[:, :nsub], func=AF.Exp, scale=-0.5
        )
        wbf = wp.tile([P, 4, D], BF16, name="wbf", tag="wbf")
        for i in range(nsub):
            nc.vector.tensor_scalar(
                out=wbf[:, i, :],
                in0=wnat[:, i, :],
                scalar1=wrn[:, i : i + 1],
                scalar2=None,
                op0=ALU.mult,
            )
        # transpose to [d, c] layout (bf16 through the PE, fp8 on copy-out)
        wT = wp.tile([P, NK, CHUNK], FP8, name="wT", tag="wT")
        for k in range(NK):
            tpw = pst.tile([P, 4, P], BF16, name="tp", tag="tp")
            for i in range(nsub):
                nc.tensor.transpose(tpw[:, i, :], wbf[:, i, ts(k, P)], ident[:])
            nc.vector.tensor_copy(
                out=wT[:, k, :csz],
                in_=tpw[:, :nsub, :].rearrange("p a b -> p (a b)"),
            )
        # matmuls (fp8 DoubleRow: K=256 per op) + fused exp/rowsum
        for t in range(NBT):
            ps = psm.tile([P, CHUNK], F32, name="ps", tag="ps")
            for kp in range(0, NK, 2):
                nc.tensor.matmul(
                    ps[:, :csz],
                    lhsT=fT[:, kp : kp + 2, ts(t, P)],
                    rhs=wT[:, kp : kp + 2, :csz],
                    start=(kp == 0),
                    stop=(kp == NK - 2),
                    perf_mode=mybir.MatmulPerfMode.DoubleRow,
                )
            nc.scalar.activation(
                out=ps[:, :csz],
                in_=ps[:, :csz],
                func=AF.Exp,
                scale=SCALE,
                accum_out=srows[:, t * NCHUNK + ci : t * NCHUNK + ci + 1],
            )


def build_nc(bench_reps=0):
    """bench_reps>0 wraps the (idempotent) main class-chunk loop in a hardware
    For_i that repeats it, for timing purposes; the result stays correct."""
    import os
    from contextlib import nullcontext

    dbg_skip = set(os.environ.get("AF_SKIP", "").split(","))
    nc = bacc.Bacc("TRN2", target_bir_lowering=False, debug=False, num_devices=NCORE)

    feat = nc.dram_tensor("features", [B, D], F32, kind="ExternalInput")
    lab = nc.dram_tensor("labels_local", [B], I32, kind="ExternalInput")
    wsh = nc.dram_tensor("weight_shard", [CS_PAD, D], F32, kind="ExternalInput")
    out = nc.dram_tensor("out", [1, 1], F32, kind="ExternalOutput")
    cnt_out = (
        nc.dram_tensor("cnt_out", [1, 1], F32, kind="ExternalOutput")
        if bench_reps > 0
        else None
    )

    with tile.TileContext(nc) as tc:
        with (
            tc.tile_pool(name="persist", bufs=1) as pp,
            tc.tile_pool(name="work", bufs=2) as wp,
            tc.tile_pool(name="wdma", bufs=3) as wd,
            tc.tile_pool(name="psmm", bufs=4, space="PSUM") as psm,
            tc.tile_pool(name="pstr", bufs=2, space="PSUM") as pst,
            tc.tile_pool(name="psmisc", bufs=1, space="PSUM") as psc,
            tc.tile_pool(name="dram", bufs=1, space="DRAM") as dp,
        ):
            # ---------------- constants ----------------
            ident = pp.tile([P, P], BF16, name="ident", tag="ident")
            make_identity(nc, ident[:])
            identf = pp.tile([P, P], F32, name="identf", tag="identf")
            make_identity(nc, identf[:])
            ones_col = pp.tile([P, 1], F32, name="ones_col", tag="ones_col")
            nc.vector.memset(ones_col[:], 1.0)
            negsm = pp.tile([P, 1], F32, name="negsm", tag="negsm")
            nc.vector.memset(negsm[:], -SM)

            # ---------------- feature preprocessing ----------------
            # load f as [p, t, d] (row i = t*128+p)
            fnat = pp.tile([P, NBT, D], F32, name="fnat", tag="fnat")
            nc.sync.dma_start(
                out=fnat[:], in_=feat[:, :].rearrange("(t p) d -> p t d", p=P)
            )
            fn2 = pp.tile([P, NBT], F32, name="fn2", tag="fn2")
            for t in range(NBT):
                fsq = wp.tile([P, D], F32, name="fsq", tag="sqdump")
                nc.vector.scalar_tensor_tensor(
                    out=fsq[:],
                    in0=fnat[:, t, :],
                    scalar=1.0,
                    in1=fnat[:, t, :],
                    op0=ALU.mult,
                    op1=ALU.mult,
                    accum_out=fn2[:, t : t + 1],
                )
            frn = pp.tile([P, NBT], F32, name="frn", tag="frn")
            nc.vector.tensor_scalar_add(frn[:], fn2[:], 1e-24)
            nc.scalar.activation(out=frn[:], in_=frn[:], func=AF.Ln)
            nc.scalar.activation(out=frn[:], in_=frn[:], func=AF.Exp, scale=-0.5)

            # normalized f (f32), natural layout — kept for the label-dot
            fnorm = pp.tile([P, NBT, D], F32, name="fnorm", tag="fnorm")
            for t in range(NBT):
                nc.vector.tensor_scalar(
                    out=fnorm[:, t, :],
                    in0=fnat[:, t, :],
                    scalar1=frn[:, t : t + 1],
                    scalar2=None,
                    op0=ALU.mult,
                )

            # fT[k][d=128, b=1024] — stationary operand for the matmuls
            fT = pp.tile([P, NK, B], FP8, name="fT", tag="fT")
            for k in range(NK):
                for tg in range(2):
                    tpf = pst.tile([P, 4, P], F32, name="tpf", tag="tpf", bufs=1)
                    for j in range(4):
                        t = tg * 4 + j
                        nc.tensor.transpose(
                            tpf[:, j, :],
                            fnorm[:, t, ts(k, P)],
                            identf[:],
                        )
                    nc.vector.tensor_copy(
                        out=fT[:, k, tg * 512 : (tg + 1) * 512],
                        in_=tpf[:].rearrange("p a b -> p (a b)"),
                    )

            # ---------------- label path ----------------
            labs = pp.tile([P, NBT], I32, name="labs", tag="labs")
            nc.sync.dma_start(out=labs[:], in_=lab[:].rearrange("(t p) -> p t", p=P))
            labf = pp.tile([P, NBT], F32, name="labf", tag="labf")
            nc.vector.tensor_copy(out=labf[:], in_=labs[:])
            clampf = pp.tile([P, NBT], F32, name="clampf", tag="clampf")
            nc.vector.tensor_scalar(
                out=clampf[:],
                in0=labf[:],
                scalar1=0.0,
                scalar2=float(CS - 1),
                op0=ALU.max,
                op1=ALU.min,
            )
            idx = pp.tile([P, NBT], I32, name="idx", tag="idx")
            nc.vector.tensor_copy(out=idx[:], in_=clampf[:])
            mge = wp.tile([P, NBT], F32, name="mge", tag="mge")
            nc.vector.tensor_scalar(
                out=mge[:], in0=labf[:], scalar1=0.0, scalar2=None, op0=ALU.is_ge
            )
            mle = wp.tile([P, NBT], F32, name="mle", tag="mle")
            nc.vector.tensor_scalar(
                out=mle[:],
                in0=labf[:],
                scalar1=float(CS - 1),
                scalar2=None,
                op0=ALU.is_le,
            )
            mask = pp.tile([P, NBT], F32, name="mask", tag="mask")
            nc.vector.tensor_tensor(
                out=mask[:], in0=mge[:], in1=mle[:], op=ALU.mult
            )

            gdot = pp.tile([P, NBT], F32, name="gdot", tag="gdot")
            wln2 = pp.tile([P, NBT], F32, name="wln2", tag="wln2")
            for t in range(NBT):
                wlab = wp.tile([P, D], F32, name="wlab", tag="wlab")
                if "gather" in dbg_skip:
                    nc.vector.memset(wlab[:], 1.0)
                else:
                    nc.gpsimd.indirect_dma_start(
                        out=wlab[:],
                        out_offset=None,
                        in_=wsh[:, :],
                        in_offset=bass.IndirectOffsetOnAxis(
                            ap=idx[:, t : t + 1], axis=0
                        ),
                    )
                dump = wp.tile([P, D], F32, name="dump", tag="sqdump")
                nc.vector.scalar_tensor_tensor(
                    out=dump[:],
                    in0=wlab[:],
                    scalar=1.0,
                    in1=wlab[:],
                    op0=ALU.mult,
                    op1=ALU.mult,
                    accum_out=wln2[:, t : t + 1],
                )
                dump2 = wp.tile([P, D], F32, name="dump2", tag="sqdump")
                nc.vector.scalar_tensor_tensor(
                    out=dump2[:],
                    in0=wlab[:],
                    scalar=1.0,
                    in1=fnorm[:, t, :],
                    op0=ALU.mult,
                    op1=ALU.mult,
                    accum_out=gdot[:, t : t + 1],
                )
            wlrn = pp.tile([P, NBT], F32, name="wlrn", tag="wlrn")
            nc.vector.tensor_scalar_add(wlrn[:], wln2[:], 1e-24)
            nc.scalar.activation(out=wlrn[:], in_=wlrn[:], func=AF.Ln)
            nc.scalar.activation(out=wlrn[:], in_=wlrn[:], func=AF.Exp, scale=-0.5)

            # g0 = cos at label = gdot * wlrn ; logits use scale 64 in ACT
            g0 = pp.tile([P, NBT], F32, name="g0", tag="g0")
            nc.vector.tensor_tensor(out=g0[:], in0=gdot[:], in1=wlrn[:], op=ALU.mult)
            e1 = wp.tile([P, NBT], F32, name="e1", tag="e1")
            nc.scalar.activation(out=e1[:], in_=g0[:], func=AF.Exp, scale=SCALE)
            e0 = wp.tile([P, NBT], F32, name="e0", tag="e0")
            nc.scalar.activation(
                out=e0[:], in_=g0[:], func=AF.Exp, scale=SCALE, bias=negsm[:, :1]
            )

            arbuf = pp.tile([P, 24], F32, name="arbuf", tag="arbuf")
            d0 = wp.tile([P, NBT], F32, name="d0", tag="d0")
            nc.vector.tensor_tensor(out=d0[:], in0=e0[:], in1=e1[:], op=ALU.subtract)
            nc.vector.tensor_tensor(
                out=arbuf[:, 8:16], in0=d0[:], in1=mask[:], op=ALU.mult
            )
            tgt0 = wp.tile([P, NBT], F32, name="tgt0", tag="tgt0")
            nc.vector.tensor_scalar(
                out=tgt0[:],
                in0=g0[:],
                scalar1=SCALE,
                scalar2=-SM,
                op0=ALU.mult,
                op1=ALU.add,
            )
            nc.vector.tensor_tensor(
                out=arbuf[:, 16:24], in0=tgt0[:], in1=mask[:], op=ALU.mult
            )

            # ---------------- main loop over class chunks ----------------
            srows = pp.tile([P, NBT * NCHUNK], F32, name="srows", tag="srows")
            if "main" in dbg_skip:
                nc.vector.memset(srows[:], 1.0)
            n_chunks_used = 0 if "main" in dbg_skip else NCHUNK
            if bench_reps > 0:
                cnt = pp.tile([1, 1], F32, name="cnt", tag="cnt")
                nc.vector.memset(cnt[:], 0.0)
            loop_cm = tc.For_i(0, bench_reps, 1) if bench_reps > 0 else nullcontext()
            with loop_cm:
                main_chunk_loop(
                    nc, tc, wd, wp, pst, psm, wsh, fT, ident, srows, n_chunks_used
                )
                if bench_reps > 0:
                    nc.vector.tensor_scalar_add(cnt[:], cnt[:], 1.0)
            if bench_reps > 0:
                nc.sync.dma_start(out=cnt_out[:, :], in_=cnt[:])

            # reduce srows over chunks -> S per b-tile, into arbuf cols 0:8
            nc.vector.tensor_reduce(
                out=arbuf[:, 0:8],
                in_=srows[:].rearrange("p (t c) -> p t c", c=NCHUNK),
                axis=mybir.AxisListType.X,
                op=ALU.add,
            )

            # ---------------- all-reduce ----------------
            cc_in = dp.tile([P, 24], F32, name="cc_in", tag="cc_in")
            cc_out = dp.tile([P, 24], F32, name="cc_out", tag="cc_out")
            nc.sync.dma_start(out=cc_in[:], in_=arbuf[:])
            nc.gpsimd.collective_compute(
                "AllReduce",
                ALU.add,
                replica_groups=[list(range(NCORE))],
                ins=[cc_in[:].opt()],
                outs=[cc_out[:].opt()],
            )
            red = pp.tile([P, 24], F32, name="red", tag="red")
            nc.sync.dma_start(out=red[:], in_=cc_out[:])

            # ---------------- final loss ----------------
            zb = wp.tile([P, NBT], F32, name="zb", tag="zb")
            nc.vector.tensor_tensor(
                out=zb[:], in0=red[:, 0:8], in1=red[:, 8:16], op=ALU.add
            )
            lz = wp.tile([P, NBT], F32, name="lz", tag="lz")
            nc.scalar.activation(out=lz[:], in_=zb[:], func=AF.Ln)
            lmt = wp.tile([P, NBT], F32, name="lmt", tag="lmt")
            nc.vector.tensor_tensor(
                out=lmt[:], in0=lz[:], in1=red[:, 16:24], op=ALU.subtract
            )
            rs = wp.tile([P, 1], F32, name="rs", tag="rs")
            nc.vector.tensor_reduce(
                out=rs[:], in_=lmt[:], axis=mybir.AxisListType.X, op=ALU.add
            )
            lps = psc.tile([1, 1], F32, name="lps", tag="lps")
            nc.tensor.matmul(lps[:], lhsT=ones_col[:], rhs=rs[:], start=True, stop=True)
            osb = wp.tile([1, 1], F32, name="osb", tag="osb")
            nc.scalar.mul(osb[:], lps[:], 1.0 / B)
            nc.sync.dma_start(out=out[:, :], in_=osb[:])

    nc.compile()
    return nc


_NC_CACHE = None


def _get_nc():
    global _NC_CACHE
    if _NC_CACHE is None:
        _NC_CACHE = build_nc()
    return _NC_CACHE


def _make_in_maps(features, labels, weight):
    feats = np.ascontiguousarray(np.asarray(features, dtype=np.float32))
    w = np.asarray(weight, dtype=np.float32)
    labs = np.asarray(labels).astype(np.int64)
    wpad = np.zeros((NCORE, CS_PAD, D), dtype=np.float32)
    wpad[:, :CS, :] = w.reshape(NCORE, CS, D)
    return [
        {
            "features": feats,
            "labels_local": (labs - i * CS).astype(np.int32),
            "weight_shard": np.ascontiguousarray(wpad[i]),
        }
        for i in range(NCORE)
    ]


def run_spmd(features, labels, weight, trace=False):
    """Returns (loss_scalar, BassKernelResults)."""
    from concourse.bass_utils import run_bass_kernel_spmd

    in_maps = _make_in_maps(features, labels, weight)
    res = run_bass_kernel_spmd(
        _get_nc(), in_maps, core_ids=list(range(NCORE)), trace=trace
    )
    loss = np.float32(res.results[0]["out"].reshape(())[()])
    return loss, res


def kernel(features, labels, weight):
    loss, _ = run_spmd(features, labels, weight, trace=False)
    return np.asarray(loss, dtype=np.float32).reshape(())

